# revision 1
# baseline (speedup 1.0000x reference)
"""Trainium2 Bass kernel for the DIRU gated multi-compartment RNN.

Model (per timestep t, scan over T):
    rec    = h @ W_rec.T + b_rec                  # [B, K*H]
    inp    = einsum('bi,khi->bkh', x_t, W_in)+b_in# [B, K, H]
    outs   = tanh(inp + rec)                      # [B, K, H]
    logits = outs.reshape(B,K*H) @ W_gate.T + b_g # [B, K]
    w      = softmax(logits, axis=1)
    h      = sum_k outs[:,k,:] * w[:,k,None]      # [B, H]
final: y = h @ W_fc.T + b_fc                      # [B, O]

Sharding: data-parallel over batch B=1024 across 8 cores -> 128 rows/core,
which exactly fills the 128 SBUF partitions. Weights replicated.

Per-core layout strategy ("T-hybrid"):
  * q = rec+inp accumulated in PSUM in T-layout [j=K*H on partitions
    (8 chunks of 128), b on free dim].  Biases folded into the matmuls
    (beta row via a ones-row appended to x^T).
  * tanh: one ACT instruction PSUM->SBUF keeps T-layout (outsT), which
    directly feeds the logits matmuls (lhsT must be SBUF).
  * logits -> PSUM [b, 4] (B-layout): softmax via ACT exp with accum_out
    (row sum), DVE reciprocal, per-partition scalar multiply.
  * PE transposes bridge outsT -> outs B-layout in PSUM for the gating,
    which uses per-partition-scalar fused multiply-add (scalar_tensor_tensor).
  * h is re-transposed (PE) to T-layout for the next step's rec matmuls.
x is pre-transposed on the host to [T, I, B_local] so no x transposes are
needed on-chip.
"""

import numpy as np

import concourse.bacc as bacc
import concourse.bass as bass
import concourse.tile as tile
from concourse import mybir
from concourse.bass_utils import run_bass_kernel_spmd

B, T, I, H, K, O = 1024, 512, 40, 256, 4, 16
NCORES = 8
BL = B // NCORES          # 128 batch rows per core
KH = K * H                # 1024
NJC = KH // 128           # 8 j-chunks of 128
F32 = mybir.dt.float32


def build_nc(t_steps: int = T, use_beta: bool = False, use_bg: bool = False):
    nc = bacc.Bacc(None, target_bir_lowering=False, debug=True)

    xT = nc.dram_tensor("xT", [t_steps, I, BL], F32, kind="ExternalInput")
    wrecT = nc.dram_tensor("wrecT", [2, 128, KH], F32, kind="ExternalInput")
    wiaug = nc.dram_tensor("wiaug", [I, KH], F32, kind="ExternalInput")
    beta = nc.dram_tensor("beta", [1, KH], F32, kind="ExternalInput")
    wgT = nc.dram_tensor("wgT", [128, NJC, K], F32, kind="ExternalInput")
    bg = nc.dram_tensor("bg", [1, K], F32, kind="ExternalInput")
    wfcT = nc.dram_tensor("wfcT", [2, 128, O], F32, kind="ExternalInput")
    bfc = nc.dram_tensor("bfc", [1, O], F32, kind="ExternalInput")
    ident = nc.dram_tensor("ident", [128, 128], F32, kind="ExternalInput")
    y = nc.dram_tensor("y", [BL, O], F32, kind="ExternalOutput")

    mult = mybir.AluOpType.mult
    add = mybir.AluOpType.add
    AF = mybir.ActivationFunctionType

    with tile.TileContext(nc) as tc:
        with (
            tc.tile_pool(name="const", bufs=1) as const,
            tc.tile_pool(name="xa", bufs=4) as xpool,
            tc.tile_pool(name="state", bufs=2) as spool,
            tc.tile_pool(name="work", bufs=2) as wpool,
            tc.tile_pool(name="qp", bufs=2, space="PSUM") as qp,
            tc.tile_pool(name="lg", bufs=1, space="PSUM") as lgp,
            tc.tile_pool(name="ob", bufs=1, space="PSUM") as obp,
            tc.tile_pool(name="htp", bufs=1, space="PSUM") as htpp,
        ):
            # ---- constants into SBUF ----
            sb_wrecT = const.tile([128, 2, KH], F32)
            for cc in range(2):
                nc.sync.dma_start(out=sb_wrecT[:, cc, :], in_=wrecT[cc])
            sb_wiaug = const.tile([I, KH], F32)
            nc.sync.dma_start(out=sb_wiaug, in_=wiaug[:, :])
            sb_beta = const.tile([1, KH], F32)
            nc.sync.dma_start(out=sb_beta, in_=beta[:, :])
            sb_wgT = const.tile([128, NJC, K], F32)
            nc.sync.dma_start(out=sb_wgT, in_=wgT[:, :, :])
            sb_bg = const.tile([1, K], F32)
            nc.sync.dma_start(out=sb_bg, in_=bg[:, :])
            sb_wfcT = const.tile([128, 2, O], F32)
            for cc in range(2):
                nc.sync.dma_start(out=sb_wfcT[:, cc, :], in_=wfcT[cc])
            sb_bfc = const.tile([1, O], F32)
            nc.sync.dma_start(out=sb_bfc, in_=bfc[:, :])
            sb_ident = const.tile([128, 128], F32)
            nc.sync.dma_start(out=sb_ident, in_=ident[:, :])
            sb_ones = const.tile([1, 128], F32)
            nc.vector.memset(sb_ones, 1.0)

            def load_x(t):
                xa = xpool.tile([I, BL], F32, tag="xa")
                nc.sync.dma_start(out=xa, in_=xT[t])
                return xa

            def inp_mms(qt, xa, only_group_member):
                # q[j, b] += sum_i wiaug[i, j] * xa[i, b]  (+ beta if nonzero)
                # start=True only on the first matmul touching each PSUM bank:
                # the accumulate-bit clear is bank-wide, so a second start=True
                # in the same bank would wipe sibling chunks' has_written bits.
                for jc in range(NJC):
                    nc.tensor.matmul(
                        qt[:, bass.ts(jc, 128)],
                        lhsT=sb_wiaug[:, bass.ts(jc, 128)],
                        rhs=xa,
                        start=(jc % 4 == 0),
                        stop=(only_group_member and not use_beta),
                    )
                    if use_beta:
                        nc.tensor.matmul(
                            qt[:, bass.ts(jc, 128)],
                            lhsT=sb_beta[:, bass.ts(jc, 128)],
                            rhs=sb_ones,
                            start=False,
                            stop=only_group_member,
                        )

            xa_next = load_x(0)
            qt_next = qp.tile([128, KH], F32, tag="q")
            inp_mms(qt_next, xa_next, True)  # t=0 has no rec matmuls (h0 = 0)

            hT = None
            for t in range(t_steps):
                qt = qt_next
                if t + 1 < t_steps:
                    xa_next = load_x(t + 1)
                if hT is not None:
                    # rec: q[j, b] += sum_c wrecT[c, j] * hT[c, b]
                    for jc in range(NJC):
                        for cc in range(2):
                            nc.tensor.matmul(
                                qt[:, bass.ts(jc, 128)],
                                lhsT=sb_wrecT[:, cc, bass.ts(jc, 128)],
                                rhs=hT[:, bass.ts(cc, 128)],
                                start=False,
                                stop=(cc == 1),
                            )

                outsT = wpool.tile([128, KH], F32, tag="outsT")
                nc.scalar.activation(outsT, qt, AF.Tanh)

                # logits[b, k] = sum_j outsT[j, b] * wgT[j, k]  (+ b_gate)
                lgt = lgp.tile([128, K], F32, tag="lg")
                for jc in range(NJC):
                    nc.tensor.matmul(
                        lgt,
                        lhsT=outsT[:, bass.ts(jc, 128)],
                        rhs=sb_wgT[:, jc, :],
                        start=(jc == 0),
                        stop=(jc == NJC - 1 and not use_bg),
                    )
                if use_bg:
                    nc.tensor.matmul(
                        lgt, lhsT=sb_ones, rhs=sb_bg, start=False, stop=True
                    )

                # softmax weights: g = exp(l) / sum_k exp(l)
                e_t = wpool.tile([128, K], F32, tag="e")
                z_t = wpool.tile([128, 1], F32, tag="z")
                nc.scalar.activation(e_t, lgt, AF.Exp, accum_out=z_t)
                rz = wpool.tile([128, 1], F32, tag="rz")
                nc.vector.reciprocal(rz, z_t)
                g_t = wpool.tile([128, K], F32, tag="g")
                nc.vector.tensor_scalar_mul(g_t, e_t, rz)

                # transpose outsT -> outs B-layout [b, j] in PSUM
                obt = obp.tile([128, KH], F32, tag="ob")
                for jc in range(NJC):
                    nc.tensor.transpose(
                        obt[:, bass.ts(jc, 128)],
                        outsT[:, bass.ts(jc, 128)],
                        sb_ident,
                    )

                # gating: u[b, c] = sum_k g[b, k] * outs[b, k*H + c]
                u_t = wpool.tile([128, H], F32, tag="u")
                nc.vector.tensor_scalar_mul(u_t, obt[:, 0:H], g_t[:, 0:1])
                for k in range(1, K):
                    nc.vector.scalar_tensor_tensor(
                        u_t,
                        in0=obt[:, k * H:(k + 1) * H],
                        scalar=g_t[:, k:k + 1],
                        in1=u_t,
                        op0=mult,
                        op1=add,
                    )

                # h -> T-layout for next step (or final matmul)
                htp_t = htpp.tile([128, H], F32, tag="htp")
                for cc in range(2):
                    nc.tensor.transpose(
                        htp_t[:, bass.ts(cc, 128)],
                        u_t[:, bass.ts(cc, 128)],
                        sb_ident,
                    )
                hT = spool.tile([128, H], F32, tag="hT")
                nc.vector.tensor_copy(hT, htp_t)

                # prefetch x-projection for t+1 into the other q buffer
                if t + 1 < t_steps:
                    qt_next = qp.tile([128, KH], F32, tag="q")
                    inp_mms(qt_next, xa_next, False)

            # final: y = h @ W_fc.T + b_fc
            yp = lgp.tile([128, O], F32, tag="lg")
            for cc in range(2):
                nc.tensor.matmul(
                    yp,
                    lhsT=hT[:, bass.ts(cc, 128)],
                    rhs=sb_wfcT[:, cc, :],
                    start=(cc == 0),
                    stop=False,
                )
            nc.tensor.matmul(yp, lhsT=sb_ones, rhs=sb_bfc, start=False, stop=True)
            ysb = wpool.tile([128, O], F32, tag="y")
            nc.vector.tensor_copy(ysb, yp)
            nc.sync.dma_start(out=y[:, :], in_=ysb)

    nc.compile()
    return nc


def _prep_weights(W_in, b_in, W_rec, b_rec, W_gate, b_gate, W_fc, b_fc):
    W_in = np.asarray(W_in, np.float32)
    b_in = np.asarray(b_in, np.float32)
    W_rec = np.asarray(W_rec, np.float32)
    b_rec = np.asarray(b_rec, np.float32)
    W_gate = np.asarray(W_gate, np.float32)
    b_gate = np.asarray(b_gate, np.float32)
    W_fc = np.asarray(W_fc, np.float32)
    b_fc = np.asarray(b_fc, np.float32)

    # wiaug[i, j] = W_in[k, h, i] with j = k*H + h ; beta[j] = b_in + b_rec
    wiaug = np.ascontiguousarray(W_in.reshape(KH, I).T)
    beta = (b_in.reshape(KH) + b_rec).reshape(1, KH)
    # wrecT[cc, p, j] = W_rec[j, cc*128 + p]
    wrecT = W_rec.T.reshape(2, 128, KH).copy()
    # wgT[p, jc, k] = W_gate[k, jc*128 + p]
    wgT = np.ascontiguousarray(W_gate.T.reshape(NJC, 128, K).transpose(1, 0, 2))
    # wfcT[cc, p, o] = W_fc[o, cc*128 + p]
    wfcT = W_fc.T.reshape(2, 128, O).copy()
    return {
        "wiaug": wiaug,
        "beta": beta,
        "wrecT": wrecT,
        "wgT": wgT,
        "bg": b_gate.reshape(1, K),
        "wfcT": wfcT,
        "bfc": b_fc.reshape(1, O),
        "ident": np.eye(128, dtype=np.float32),
    }


_NC_CACHE: dict = {}


def get_cached_nc(key=None):
    if key is None:
        return next(iter(_NC_CACHE.values()))
    return _NC_CACHE.get(key)


def kernel(x, W_in, b_in, W_rec, b_rec, W_gate, b_gate, W_fc, b_fc, **run_kwargs):
    x = np.asarray(x, np.float32)
    t_steps = x.shape[1]
    weights = _prep_weights(W_in, b_in, W_rec, b_rec, W_gate, b_gate, W_fc, b_fc)

    key = (t_steps, bool(np.any(weights["beta"])), bool(np.any(weights["bg"])))
    if key not in _NC_CACHE:
        _NC_CACHE[key] = build_nc(key[0], use_beta=key[1], use_bg=key[2])
    nc = _NC_CACHE[key]
    in_maps = []
    for c in range(NCORES):
        xs = x[c * BL:(c + 1) * BL]                     # [BL, T, I]
        xTd = np.ascontiguousarray(xs.transpose(1, 2, 0))  # [T, I, BL]
        in_maps.append({"xT": xTd, **weights})

    res = run_bass_kernel_spmd(nc, in_maps, list(range(NCORES)), **run_kwargs)
    out = np.concatenate([res.results[c]["y"] for c in range(NCORES)], axis=0)
    if run_kwargs:
        return out, res
    return out



# revision 17
# speedup vs baseline: 1.3650x; 1.3650x over previous
"""Trainium2 Bass kernel for the DIRU gated multi-compartment RNN.

Model (per timestep t, scan over T):
    rec    = h @ W_rec.T + b_rec                  # [B, K*H]
    inp    = einsum('bi,khi->bkh', x_t, W_in)+b_in# [B, K, H]
    outs   = tanh(inp + rec)                      # [B, K, H]
    logits = outs.reshape(B,K*H) @ W_gate.T + b_g # [B, K]
    w      = softmax(logits, axis=1)
    h      = sum_k outs[:,k,:] * w[:,k,None]      # [B, H]
final: y = h @ W_fc.T + b_fc                      # [B, O]

Sharding: data-parallel over batch B=1024 across 8 cores -> 128 rows/core.

Speed design (vs fp32 baseline):
  * All matmuls in bf16 (1 cycle/row vs 4 for fp32 on the PE).
  * T-layout q accumulation [j on partitions, b free]; tanh -> outsT (bf16).
  * Gating in B-layout via per-partition-scalar fused mul-add chains,
    split across DVE (c-half 0) and GPSIMD (c-half 1) to halve the serial
    chain; obt comes from 8 bf16 PE transposes done while PE is idle.
  * hT copies PSUM->SBUF on GPSIMD (tensor_copy), freeing ACT/DVE.
  * PE "heater" dummy matmuls fill PE idle windows so the tensor engine
    p-state stays at max clock (idle gaps would halve the clock).
"""

import numpy as np

import concourse.bacc as bacc
import concourse.bass as bass
import concourse.tile as tile
from concourse import mybir
from concourse.bass_utils import run_bass_kernel_spmd

B, T, I, H, K, O = 1024, 512, 40, 256, 4, 16
NCORES = 8
BL = B // NCORES          # 128 batch rows per core
KH = K * H                # 1024
NJC = KH // 128           # 8 j-chunks of 128
F32 = mybir.dt.float32
BF16 = mybir.dt.bfloat16

# gating half-chain engine split: True -> DVE does cc0, gpsimd does cc1.
POOL_GATING = True
# heater counts: dummy PE matmuls after (rec, transposes, inp) program points
HEAT = (10, 12, 4)
HEAT_F = 256  # free size of each heater matmul


def build_nc(t_steps: int = T, use_beta: bool = False, use_bg: bool = False,
             pool_gating: bool = POOL_GATING, heat: tuple = HEAT):
    nc = bacc.Bacc(None, target_bir_lowering=False, debug=True)

    xT = nc.dram_tensor("xT", [t_steps, I, BL], BF16, kind="ExternalInput")
    wrecT = nc.dram_tensor("wrecT", [2, 128, KH], BF16, kind="ExternalInput")
    wiaug = nc.dram_tensor("wiaug", [I, KH], BF16, kind="ExternalInput")
    beta = nc.dram_tensor("beta", [1, KH], BF16, kind="ExternalInput")
    wgT = nc.dram_tensor("wgT", [128, NJC, K], F32, kind="ExternalInput")
    bg = nc.dram_tensor("bg", [1, K], F32, kind="ExternalInput")
    wfcT = nc.dram_tensor("wfcT", [2, 128, O], BF16, kind="ExternalInput")
    bfc = nc.dram_tensor("bfc", [1, O], BF16, kind="ExternalInput")
    ident = nc.dram_tensor("ident", [128, 128], F32, kind="ExternalInput")
    y = nc.dram_tensor("y", [BL, O], F32, kind="ExternalOutput")

    mult = mybir.AluOpType.mult
    add = mybir.AluOpType.add
    AF = mybir.ActivationFunctionType

    with tile.TileContext(nc) as tc:
        with (
            tc.tile_pool(name="const", bufs=1) as const,
            tc.tile_pool(name="xa", bufs=3) as xpool,
            tc.tile_pool(name="state", bufs=2) as spool,
            tc.tile_pool(name="work", bufs=2) as wpool,
            tc.tile_pool(name="qp", bufs=1, space="PSUM") as qp,
            tc.tile_pool(name="lg", bufs=1, space="PSUM") as lgp,
            tc.tile_pool(name="ob", bufs=1, space="PSUM") as obp,
            tc.tile_pool(name="htp", bufs=1, space="PSUM") as htpp,
            tc.tile_pool(name="heatp", bufs=1, space="PSUM") as heatpool,
        ):
            # ---- constants into SBUF ----
            sb_wrecT = const.tile([128, 2, KH], BF16)
            for cc in range(2):
                nc.sync.dma_start(out=sb_wrecT[:, cc, :], in_=wrecT[cc])
            sb_wiaug = const.tile([I, KH], BF16)
            nc.sync.dma_start(out=sb_wiaug, in_=wiaug[:, :])
            sb_beta = const.tile([1, KH], BF16)
            nc.sync.dma_start(out=sb_beta, in_=beta[:, :])
            sb_wgT = const.tile([128, NJC, K], F32)
            nc.sync.dma_start(out=sb_wgT, in_=wgT[:, :, :])
            sb_bg = const.tile([1, K], F32)
            nc.sync.dma_start(out=sb_bg, in_=bg[:, :])
            sb_wfcT = const.tile([128, 2, O], BF16)
            for cc in range(2):
                nc.sync.dma_start(out=sb_wfcT[:, cc, :], in_=wfcT[cc])
            sb_bfc = const.tile([1, O], BF16)
            nc.sync.dma_start(out=sb_bfc, in_=bfc[:, :])
            sb_ident = const.tile([128, 128], F32)
            nc.sync.dma_start(out=sb_ident, in_=ident[:, :])
            sb_ones = const.tile([1, 128], BF16)
            nc.vector.memset(sb_ones, 1.0)
            sb_ones32 = const.tile([1, 128], F32)
            nc.vector.memset(sb_ones32, 1.0)
            # heater operands: constant bf16 tile; results go to a scratch
            # PSUM bank nothing reads.
            sb_heat = const.tile([128, HEAT_F], BF16)
            nc.vector.memset(sb_heat, 0.001)
            heat_ps = heatpool.tile([128, HEAT_F], F32, tag="heatps")

            def heater(n):
                for _ in range(n):
                    nc.tensor.matmul(
                        heat_ps, lhsT=sb_heat[:, 0:128], rhs=sb_heat,
                        start=True, stop=True, skip_group_check=True,
                    )

            def load_x(t):
                xa = xpool.tile([I, BL], BF16, tag="xa")
                nc.sync.dma_start(out=xa, in_=xT[t])
                return xa

            def inp_mms(qt, xa, only_group_member):
                # q[j, b] += sum_i wiaug[i, j] * xa[i, b]  (+ beta if nonzero)
                # start=True only on the first matmul touching each PSUM bank.
                for jc in range(NJC):
                    nc.tensor.matmul(
                        qt[:, bass.ts(jc, 128)],
                        lhsT=sb_wiaug[:, bass.ts(jc, 128)],
                        rhs=xa,
                        start=(jc % 4 == 0),
                        stop=(only_group_member and not use_beta),
                    )
                    if use_beta:
                        nc.tensor.matmul(
                            qt[:, bass.ts(jc, 128)],
                            lhsT=sb_beta[:, bass.ts(jc, 128)],
                            rhs=sb_ones,
                            start=False,
                            stop=only_group_member,
                        )

            xa_next = load_x(0)
            xa_next2 = load_x(1) if t_steps > 1 else None
            qt_next = qp.tile([128, KH], F32, tag="q")
            inp_mms(qt_next, xa_next, True)  # t=0 has no rec matmuls (h0 = 0)

            hT = None
            for t in range(t_steps):
                qt = qt_next
                xa_next = xa_next2
                if hT is not None:
                    # rec: q[j, b] += sum_c wrecT[c, j] * hT[c, b]
                    # cc0 first: its half of h lands earlier.
                    for cc in range(2):
                        for jc in range(NJC):
                            nc.tensor.matmul(
                                qt[:, bass.ts(jc, 128)],
                                lhsT=sb_wrecT[:, cc, bass.ts(jc, 128)],
                                rhs=hT[:, cc, :],
                                start=False,
                                stop=(cc == 1),
                            )
                heater(heat[0])

                outsT = wpool.tile([128, KH], F32, tag="outsT")
                nc.scalar.activation(outsT, qt, AF.Tanh)

                # logits[b, k] = sum_j outsT[j, b] * wgT[j, k]  (+ b_gate)
                lgt = lgp.tile([128, K], F32, tag="lg")
                for jc in range(NJC):
                    nc.tensor.matmul(
                        lgt,
                        lhsT=outsT[:, bass.ts(jc, 128)],
                        rhs=sb_wgT[:, jc, :],
                        start=(jc == 0),
                        stop=(jc == NJC - 1 and not use_bg),
                    )
                if use_bg:
                    nc.tensor.matmul(
                        lgt, lhsT=sb_ones32, rhs=sb_bg, start=False, stop=True
                    )

                # transpose outsT -> outs B-layout [b, j] in PSUM (fp32)
                obt = obp.tile([128, KH], F32, tag="ob")
                for jc in range(NJC):
                    nc.tensor.transpose(
                        obt[:, bass.ts(jc, 128)],
                        outsT[:, bass.ts(jc, 128)],
                        sb_ident,
                    )
                heater(heat[1])

                # softmax weights: g = exp(l) / sum_k exp(l)
                e_t = wpool.tile([128, K], F32, tag="e")
                z_t = wpool.tile([128, 1], F32, tag="z")
                nc.scalar.activation(e_t, lgt, AF.Exp, accum_out=z_t)
                rz = wpool.tile([128, 1], F32, tag="rz")
                nc.vector.reciprocal(rz, z_t)
                g_t = wpool.tile([128, K], F32, tag="g")
                nc.vector.tensor_scalar_mul(g_t, e_t, rz)

                # gating: u[b, c] = sum_k g[b, k] * outs[b, k*H + c]
                # obt free layout is (jc, b)-chunked with jc = 2k + cc, so
                # compartment k spans the contiguous slice [2k*128, 2k*128+256).
                u_f = wpool.tile([128, 256], F32, tag="u")
                nc.vector.tensor_scalar_mul(u_f, obt[:, 0:256], g_t[:, 0:1])
                for k in range(1, K):
                    dst = u_f
                    if k == K - 1:
                        dst = wpool.tile([128, 256], F32, tag="ub")
                    nc.vector.scalar_tensor_tensor(
                        dst,
                        in0=obt[:, k * 256:(k + 1) * 256],
                        scalar=g_t[:, k:k + 1],
                        in1=u_f,
                        op0=mult,
                        op1=add,
                    )
                u_b = dst

                # h -> T-layout for next step (or final matmul)
                htp_t = htpp.tile([128, 2, 128], F32, tag="htp")
                for cc in range(2):
                    nc.tensor.transpose(
                        htp_t[:, cc, :], u_b[:, bass.ts(cc, 128)], sb_ident
                    )
                hT = spool.tile([128, 2, 128], BF16, tag="hT")
                nc.scalar.copy(hT[:, 0, :], htp_t[:, 0, :])
                nc.vector.tensor_copy(hT[:, 1, :], htp_t[:, 1, :])

                # prefetch x and the x-projection for t+1 into the other buffer
                if t + 2 < t_steps:
                    xa_next2 = load_x(t + 2)
                if t + 1 < t_steps:
                    qt_next = qp.tile([128, KH], F32, tag="q")
                    inp_mms(qt_next, xa_next, False)
                heater(heat[2])

            # final: y = h @ W_fc.T + b_fc
            yp = lgp.tile([128, O], F32, tag="lg")
            for cc in range(2):
                nc.tensor.matmul(
                    yp,
                    lhsT=hT[:, cc, :],
                    rhs=sb_wfcT[:, cc, :],
                    start=(cc == 0),
                    stop=False,
                )
            nc.tensor.matmul(yp, lhsT=sb_ones, rhs=sb_bfc, start=False, stop=True)
            ysb = wpool.tile([128, O], F32, tag="y")
            nc.vector.tensor_copy(ysb, yp)
            nc.sync.dma_start(out=y[:, :], in_=ysb)

    nc.compile()
    return nc


def _prep_weights(W_in, b_in, W_rec, b_rec, W_gate, b_gate, W_fc, b_fc):
    W_in = np.asarray(W_in, np.float32)
    b_in = np.asarray(b_in, np.float32)
    W_rec = np.asarray(W_rec, np.float32)
    b_rec = np.asarray(b_rec, np.float32)
    W_gate = np.asarray(W_gate, np.float32)
    b_gate = np.asarray(b_gate, np.float32)
    W_fc = np.asarray(W_fc, np.float32)
    b_fc = np.asarray(b_fc, np.float32)

    def bf(a):
        import ml_dtypes
        return np.ascontiguousarray(a).astype(ml_dtypes.bfloat16)

    # wiaug[i, j] = W_in[k, h, i] with j = k*H + h ; beta[j] = b_in + b_rec
    wiaug = np.ascontiguousarray(W_in.reshape(KH, I).T)
    beta = (b_in.reshape(KH) + b_rec).reshape(1, KH)
    # wrecT[cc, p, j] = W_rec[j, cc*128 + p]
    wrecT = W_rec.T.reshape(2, 128, KH).copy()
    # wgT[p, jc, k] = W_gate[k, jc*128 + p]
    wgT = np.ascontiguousarray(W_gate.T.reshape(NJC, 128, K).transpose(1, 0, 2))
    # wfcT[cc, p, o] = W_fc[o, cc*128 + p]
    wfcT = W_fc.T.reshape(2, 128, O).copy()
    return {
        "wiaug": bf(wiaug),
        "beta": bf(beta),
        "wrecT": bf(wrecT),
        "wgT": np.ascontiguousarray(wgT, np.float32),
        "bg": b_gate.reshape(1, K).astype(np.float32),
        "wfcT": bf(wfcT),
        "bfc": bf(b_fc.reshape(1, O)),
        "ident": np.eye(128, dtype=np.float32),
    }


_NC_CACHE: dict = {}


def get_cached_nc(key=None):
    if key is None:
        return next(iter(_NC_CACHE.values()))
    return _NC_CACHE.get(key)


def kernel(x, W_in, b_in, W_rec, b_rec, W_gate, b_gate, W_fc, b_fc, **run_kwargs):
    import ml_dtypes
    x = np.asarray(x, np.float32)
    t_steps = x.shape[1]
    weights = _prep_weights(W_in, b_in, W_rec, b_rec, W_gate, b_gate, W_fc, b_fc)

    key = (t_steps, bool(np.any(weights["beta"])), bool(np.any(weights["bg"])))
    if key not in _NC_CACHE:
        _NC_CACHE[key] = build_nc(key[0], use_beta=key[1], use_bg=key[2])
    nc = _NC_CACHE[key]
    in_maps = []
    for c in range(NCORES):
        xs = x[c * BL:(c + 1) * BL]                     # [BL, T, I]
        xTd = np.ascontiguousarray(xs.transpose(1, 2, 0)).astype(ml_dtypes.bfloat16)
        in_maps.append({"xT": xTd, **weights})

    res = run_bass_kernel_spmd(nc, in_maps, list(range(NCORES)), **run_kwargs)
    out = np.concatenate([res.results[c]["y"] for c in range(NCORES)], axis=0)
    if run_kwargs:
        return out, res
    return out
